# revision 51
# baseline (speedup 1.0000x reference)
"""Trainium2 Bass kernel: memory-slot cross-attention (nn_LocalConstructorMulti).

Reference computation (per batch b):
    Q  = memory_slots @ Wq.T                      [slots, BD]    (shared over b)
    K  = hs_b @ Wk.T                              [S, BD]
    V  = hs_b @ Wv.T                              [S, BD]
    s  = (Q_h . K_h) / sqrt(HD)  + mask           [heads, slots, S]
    p  = softmax(s, axis=S)
    o  = p @ V_h                                  [heads, slots, HD]
    y  = concat_h(o) @ Wo.T                       [slots, HID]

Sharding: 8 cores = 4 batches x 2 sequence-halves.  Masked-out rows are
compacted away on the host first (they contribute exactly zero), the
surviving rows are padded to 2*s_half and split between the batch's two
cores.  Each core computes all 8 heads over its rows and returns the
UNNORMALIZED per-head attention output plus the softmax partial sums
(the ones-column trick); the host adds the two halves, divides, and
applies the (tiny, 67 MFLOP total) o_proj in f32.

Device layout notes:
  - hs arrives pre-transposed [HID, rows] as TWO scaled fp8 e4m3 words
    per element (hs*4 ~ a8 + b8, same scale, b8 = quantized residual).
    All projections run fp8 DoubleRow matmuls at 2x PE throughput: K
    uses a8 alone (fp8 noise on K only perturbs softmax logits), V
    accumulates a8@c8 + a8@d8 + b8@c8 against the dual-word wv -- 0.75x
    the bf16 matmul time at BETTER than bf16 accuracy (two e4m3 words
    carry more mantissa than bf16).
  - hs is streamed in 512-col blocks (quads of 128-row chunks, keeping
    fp8 DRAM lines >= 512B) covering all 32 k-tiles in a few DMA
    instructions: the DGE charges a fixed ~625ns per DMA instruction, so
    per-(chunk,k)-tile DMAs would serialize and starve PE; startup
    weight/hs transfers are split into k-range pieces so the first kt
    matmuls issue ~2us in.  The 128-row granularity keeps s_half padding
    waste under ~6% (1152 vs 1280 for a 2108-row batch).
  - K is built as KT [512, rows] (bd on partitions) for the Q.K matmuls.
    Per quad, all 8 PSUM accumulation chains open at once (a start=True
    matmul marks its whole 2KB bank pending-zero, so each chain owns a
    bank: 2 kt tags x2 buffers + 4 single-buffered per-chunk V chains)
    and k runs outermost, so the PE consumes each arriving k-range piece
    of every input stream immediately -- the whole startup transfer
    window is covered by useful matmuls.
  - scores are built transposed, sT [rows, heads*slots], so the additive
    sequence mask is a per-partition bias fused into the Exp activation,
    whose scale folds out the fp8 quantization factors.
  - softmax partial sums come for free: V tiles carry an extra ones
    column, so o_psum[:, 64] accumulates sum(p); normalization happens
    on the host after combining the two row-halves.
"""

import sys

if "/opt/trn_rl_repo" not in sys.path:
    sys.path.insert(0, "/opt/trn_rl_repo")

import ml_dtypes
import numpy as np

import concourse.bass as bass  # noqa: F401  (AP helpers)
import concourse.mybir as mybir
import concourse.tile as tile
from concourse import bacc
from concourse.bass_utils import run_bass_kernel_spmd

BF16 = mybir.dt.bfloat16
F32 = mybir.dt.float32
FP8 = mybir.dt.float8e4
npbf16 = ml_dtypes.bfloat16
npe4 = ml_dtypes.float8_e4m3

B, S, HID = 4, 4096, 4096
SLOTS, HEADS, BD = 8, 8, 512
HD = BD // HEADS  # 64
N_CORES = 8
HALVES = N_CORES // B  # sequence halves per batch
MASK_NEG = -30000.0
SCALE = 1.0 / float(np.sqrt(HD))
# K-projection fp8 scaling: hs*SH and Wk*SW into e4m3's sweet spot; the
# product carries SH*SW, divided back out by the exp activation's scale.
SH = 4.0
SW = 16.0
KINV = 1.0 / (SH * SW)

# test.py can flip this to capture an NTFF profile; harness never touches it.
TRACE = False
TRACE_CORES = None
LAST_RESULT = None
# debug toggle: False disables mask compaction (full-length sequence)
COMPACT = True

_cache = {}


def _build_module(hid, s_half, chunk=128):
    """Emit + compile the single-core Bass module (same NEFF on all cores)."""
    nk = hid // 128  # contraction k-tiles
    nrt = s_half // 128  # 128-row tiles of this core's rows
    nch = s_half // chunk  # row chunks for the K/V projection
    jsub = chunk // 128  # 128-row subtiles per chunk
    nm2 = BD // 128  # kt 128-sliced bd tiles
    kb = 8  # k-tiles per hs DMA instruction
    nkb = nk // kb

    nc = bacc.Bacc("TRN2", target_bir_lowering=False, debug=False, num_devices=N_CORES)

    # hs ships as TWO scaled fp8 e4m3 words per element, hs*SH ~ a8 + b8
    # (b8 = quantized residual, same scale).  The K path consumes a8 alone;
    # the V path consumes both against the dual-word wv for bf16-level
    # accuracy at fp8 DoubleRow throughput.
    hs8T = nc.dram_tensor("hs8T", [hid, s_half], FP8, kind="ExternalInput").ap()
    hsb8T = nc.dram_tensor("hsb8T", [hid, s_half], FP8, kind="ExternalInput").ap()
    # weights arrive pre-interleaved in the SBUF layout [ki, ko*n] so their
    # DMAs are 128 straight 16KB descriptor lines.
    wk8 = nc.dram_tensor("wk8", [128, nk * BD], FP8, kind="ExternalInput").ap()
    wvc8 = nc.dram_tensor("wvc8", [128, nk * BD], FP8, kind="ExternalInput").ap()
    wvd8 = nc.dram_tensor("wvd8", [128, nk * BD], FP8, kind="ExternalInput").ap()
    # Q = memory_slots @ Wq.T is 16 MFLOP -- computed on the host in f32,
    # shipped pre-transposed/pre-scaled.
    qtH = nc.dram_tensor("qtH", [128, nm2 * SLOTS], BF16, kind="ExternalInput").ap()
    mbT = nc.dram_tensor("mbT", [128, nrt], F32, kind="ExternalInput").ap()
    # unnormalized per-head output + softmax partial sums (ones column)
    ocD = nc.dram_tensor(
        "oc", [SLOTS, HEADS * (HD + 1)], F32, kind="ExternalOutput"
    ).ap()

    hs8R = hs8T.rearrange("(ko ki) s -> ki ko s", ki=128)
    hsb8R = hsb8T.rearrange("(ko ki) s -> ki ko s", ki=128)

    with tile.TileContext(nc) as tc:
        with (
            tc.tile_pool(name="consts", bufs=1) as consts,
            tc.tile_pool(name="hsp", bufs=2) as hsp,
        ):
            # ---- resident weights / tables -------------------------------
            # All five startup streams (wk8, a8, wvc8, wvd8, b8) are split
            # into k-range pieces and interleaved per-kb: the k-outer fused
            # sweep below consumes each kb of every stream as it lands, so
            # the PE is fed from ~3us in instead of waiting for whole
            # tensors.  fp8 hs blocks span two compute chunks so their DRAM
            # lines stay >= 512B (sub-512B lines pay a 2x DMA latency
            # penalty).
            wk_sb = consts.tile([128, nk, BD], FP8)
            wkR = wk8.rearrange("p (ko n) -> p ko n", n=BD)
            wvc_sb = consts.tile([128, nk, BD], FP8)
            wvcR = wvc8.rearrange("p (ko n) -> p ko n", n=BD)
            wvd_sb = consts.tile([128, nk, BD], FP8)
            wvdR = wvd8.rearrange("p (ko n) -> p ko n", n=BD)
            span0 = min(4 * chunk, s_half)
            hs8_first = hsp.tile(
                [128, nk, 4 * chunk], FP8, tag="hs8", name="hs8_first"
            )
            hsb_first = hsp.tile(
                [128, nk, 4 * chunk], FP8, tag="hsb", name="hsb_first"
            )
            for b0 in range(nkb):
                kslc = slice(b0 * kb, (b0 + 1) * kb)
                if b0 == 0:
                    # halve the first pieces so the first matmul issues ~2us
                    # earlier (it only needs k-tiles 0..1)
                    for q in range(2):
                        qs = slice(q * kb // 2, (q + 1) * kb // 2)
                        nc.sync.dma_start(out=wk_sb[:, qs, :], in_=wkR[:, qs, :])
                        nc.sync.dma_start(
                            out=hs8_first[:, qs, 0:span0],
                            in_=hs8R[:, qs, 0:span0],
                        )
                else:
                    nc.sync.dma_start(out=wk_sb[:, kslc, :], in_=wkR[:, kslc, :])
                    nc.sync.dma_start(
                        out=hs8_first[:, kslc, 0:span0], in_=hs8R[:, kslc, 0:span0]
                    )
                nc.sync.dma_start(out=wvc_sb[:, kslc, :], in_=wvcR[:, kslc, :])
                nc.sync.dma_start(out=wvd_sb[:, kslc, :], in_=wvdR[:, kslc, :])
                nc.sync.dma_start(
                    out=hsb_first[:, kslc, 0:span0], in_=hsb8R[:, kslc, 0:span0]
                )
            qt_sb = consts.tile([128, nm2, SLOTS], BF16)  # Q.T [bd, slots]
            nc.sync.dma_start(
                out=qt_sb, in_=qtH.rearrange("p (m n) -> p m n", n=SLOTS)
            )
            mb_sb = consts.tile([128, nrt], F32)
            nc.sync.dma_start(out=mb_sb, in_=mbT)

            # ---- persistent intermediates --------------------------------
            kt_sb = consts.tile([128, nm2, s_half], BF16)  # K.T [bd, rows]
            v_sb = consts.tile([128, nrt, HEADS, HD + 1], BF16)  # V + ones col
            nc.vector.memset(v_sb[:, :, :, HD : HD + 1], 1.0)
            pt_sb = consts.tile([128, nrt, HEADS * SLOTS], BF16)  # exp(scores).T
            oc_sb = consts.tile([SLOTS, HEADS, HD + 1], F32)

            # ---- K/V projections: k-outer fused sweep --------------------
            # Per chunk pair, all 8 PSUM chains open at once (kt bd-slices
            # 0,1 pair-wide = 2 banks, V chunk x row-subtile = 4 banks, x2
            # kt buffering = 8), and k runs outermost so each arriving
            # k-range piece of every stream is consumed immediately.  kt
            # bd-slices 2,3 run as a second kt-only sweep on the resident
            # blocks.  Accumulation order within a chain is reassociated
            # (f32 PSUM, harmless).
            with tc.tile_pool(name="kvps", bufs=2, space="PSUM") as kvps:
                for q0 in range(0, nch, 4):
                    quad = [n for n in range(q0, q0 + 4) if n < nch]
                    span = min(4 * chunk, s_half - q0 * chunk)
                    if q0 == 0:
                        hs8_blk = hs8_first
                        hsb_blk = hsb_first
                    else:
                        hs8_blk = hsp.tile([128, nk, 4 * chunk], FP8, tag="hs8")
                        for b0 in range(0, nkb, 2):
                            nc.sync.dma_start(
                                out=hs8_blk[:, b0 * kb : (b0 + 2) * kb, 0:span],
                                in_=hs8R[
                                    :,
                                    b0 * kb : (b0 + 2) * kb,
                                    q0 * chunk : q0 * chunk + span,
                                ],
                            )
                        hsb_blk = hsp.tile([128, nk, 4 * chunk], FP8, tag="hsb")
                        for b0 in range(0, nkb, 2):
                            nc.sync.dma_start(
                                out=hsb_blk[:, b0 * kb : (b0 + 2) * kb, 0:span],
                                in_=hsb8R[
                                    :,
                                    b0 * kb : (b0 + 2) * kb,
                                    q0 * chunk : q0 * chunk + span,
                                ],
                            )
                    kt_ps = [
                        kvps.tile(
                            [128, 4 * chunk], F32, tag=f"kt{m}", name=f"ktA{q0}_{m}"
                        )
                        for m in range(2)
                    ]
                    v_ps = {
                        idx: kvps.tile(
                            [128, BD],
                            F32,
                            tag=f"v{idx}",
                            name=f"v{q0}_{idx}",
                            bufs=1,
                        )
                        for idx in range(len(quad))
                    }
                    vops = ((hs8_blk, wvc_sb), (hs8_blk, wvd_sb), (hsb_blk, wvc_sb))
                    for k in range(0, nk, 2):
                        st, sp = (k == 0), (k == nk - 2)
                        for m2 in range(2):
                            nc.tensor.matmul(
                                kt_ps[m2][:, 0:span],
                                wk_sb[:, k : k + 2, m2 * 128 : (m2 + 1) * 128],
                                hs8_blk[:, k : k + 2, 0:span],
                                start=st,
                                stop=sp,
                                perf_mode=mybir.MatmulPerfMode.DoubleRow,
                            )
                        for idx in range(len(quad)):
                            for pa, (blk, w_sb) in enumerate(vops):
                                nc.tensor.matmul(
                                    v_ps[idx],
                                    blk[
                                        :,
                                        k : k + 2,
                                        idx * chunk : (idx + 1) * chunk,
                                    ],
                                    w_sb[:, k : k + 2, :],
                                    start=(st and pa == 0),
                                    stop=(sp and pa == 2),
                                    perf_mode=mybir.MatmulPerfMode.DoubleRow,
                                )
                    for m2 in range(2):
                        nc.scalar.copy(
                            out=kt_sb[:, m2, q0 * chunk : q0 * chunk + span],
                            in_=kt_ps[m2][:, 0:span],
                        )
                    # second kt sweep: bd slices 2,3 on the resident blocks
                    kt_ps = [
                        kvps.tile(
                            [128, 4 * chunk], F32, tag=f"kt{m}", name=f"ktB{q0}_{m}"
                        )
                        for m in range(2)
                    ]
                    for k in range(0, nk, 2):
                        for m2 in range(2):
                            nc.tensor.matmul(
                                kt_ps[m2][:, 0:span],
                                wk_sb[:, k : k + 2, (m2 + 2) * 128 : (m2 + 3) * 128],
                                hs8_blk[:, k : k + 2, 0:span],
                                start=(k == 0),
                                stop=(k == nk - 2),
                                perf_mode=mybir.MatmulPerfMode.DoubleRow,
                            )
                    for m2 in range(2):
                        nc.scalar.copy(
                            out=kt_sb[:, m2 + 2, q0 * chunk : q0 * chunk + span],
                            in_=kt_ps[m2][:, 0:span],
                        )
                    for idx, n in enumerate(quad):
                        nc.vector.tensor_copy(
                            out=v_sb[:, n, :, 0:HD],
                            in_=v_ps[idx].rearrange("p (h d) -> p h d", h=HEADS),
                        )

            # ---- scores -> exp (all row-tiles) ---------------------------
            with tc.tile_pool(name="aps", bufs=1, space="PSUM") as aps:
                for i in range(nrt):
                    s_ps = aps.tile([128, HEADS * SLOTS], F32, tag="s", bufs=4)
                    for h in range(HEADS):
                        m2, dof = h // 2, HD * (h % 2)
                        nc.tensor.matmul(
                            s_ps[:, h * SLOTS : (h + 1) * SLOTS],
                            kt_sb[dof : dof + HD, m2, i * 128 : (i + 1) * 128],
                            qt_sb[dof : dof + HD, m2, :],
                            start=True,
                            stop=True,
                        )
                    # scale folds out the fp8 quantization factors on K
                    nc.scalar.activation(
                        out=pt_sb[:, i, :],
                        in_=s_ps,
                        func=mybir.ActivationFunctionType.Exp,
                        bias=mb_sb[:, i : i + 1],
                        scale=KINV,
                    )

            # ---- o_un = p^T @ V_aug per head -----------------------------
            # Each accumulator gets a full PSUM bank and is drained by ACT:
            # small [8,65] accumulators sharing banks with concurrently
            # DVE-read tiles fault on HW (same-bank PE-W + DVE-R erratum).
            with tc.tile_pool(name="ops", bufs=1, space="PSUM") as ops:
                for h in range(HEADS):
                    o_ps = ops.tile([128, 512], F32, tag=f"ob{h}", name=f"o_ps{h}")
                    for i in range(nrt):
                        nc.tensor.matmul(
                            o_ps[0:SLOTS, 0 : HD + 1],
                            pt_sb[:, i, h * SLOTS : (h + 1) * SLOTS],
                            v_sb[:, i, h, :],
                            start=(i == 0),
                            stop=(i == nrt - 1),
                        )
                    nc.scalar.copy(out=oc_sb[:, h, :], in_=o_ps[0:SLOTS, 0 : HD + 1])
                    if h == HEADS // 2 - 1 or h == HEADS - 1:
                        g0 = 0 if h < HEADS // 2 else HEADS // 2
                        nc.sync.dma_start(
                            out=ocD.rearrange("n (h d) -> n h d", h=HEADS)[
                                :, g0 : h + 1, :
                            ],
                            in_=oc_sb[:, g0 : h + 1, :],
                        )

    nc.compile()
    return nc


_LAST_S = S // HALVES


def _get_module(s_half=None):
    global _LAST_S
    if s_half is None:
        s_half = _LAST_S
    _LAST_S = s_half
    key = (HID, s_half)
    if key not in _cache:
        _cache[key] = _build_module(HID, s_half)
    return _cache[key]


def _prep_in_maps(hs, mask, ms, Wq, Wk, Wv, Wo):
    """Shard the full inputs into 8 per-core input maps (host-side).

    Masked-out sequence positions contribute exactly zero to the output
    (their scores get a -3e4 bias, so exp underflows to 0 and they drop
    out of both the numerator and the softmax denominator).  Compact each
    batch's unmasked rows to the front, pad to 2*s_half, and give each of
    the batch's two cores one half -- the on-device work scales with the
    unmasked count (~S/2 for a Bernoulli(1/2) mask) instead of S.  Padded
    columns are zero (K=V=0) and carry the -3e4 bias.
    """
    if COMPACT:
        idxs = [np.nonzero(mask[b])[0] for b in range(B)]
    else:
        idxs = [np.arange(S) for _ in range(B)]
    max_cnt = max(len(ix) for ix in idxs)
    s_half = min(S // 2, max(256, -(-max_cnt // 256) * 128))

    Q = (ms.astype(np.float32) @ Wq.T.astype(np.float32)) * SCALE  # [8, BD]
    qtc = np.ascontiguousarray(
        Q.T.reshape(BD // 128, 128, SLOTS).transpose(1, 0, 2).reshape(128, -1)
    ).astype(npbf16)
    nk = HID // 128

    def inter(w8):  # pre-interleave [HID, BD] into the SBUF [ki, ko*n] layout
        return np.ascontiguousarray(
            w8.reshape(nk, 128, BD).transpose(1, 0, 2).reshape(128, -1)
        )

    wk8c = inter((Wk.T * SW).astype(npe4))
    wvs = Wv.T.astype(np.float32) * SW
    wvc = wvs.astype(npe4)
    wvdc = inter((wvs - wvc.astype(np.float32)).astype(npe4))
    wvcc = inter(wvc)

    in_maps = []
    for c in range(N_CORES):
        b, r = c // HALVES, c % HALVES
        ix = idxs[b][r * s_half : (r + 1) * s_half]
        cnt = len(ix)
        g32 = hs[b][ix, :].T * SH  # [HID, cnt] f32, scaled
        a8 = np.zeros((HID, s_half), dtype=npe4)
        a8[:, :cnt] = g32.astype(npe4)
        b8 = np.zeros((HID, s_half), dtype=npe4)
        b8[:, :cnt] = (g32 - a8[:, :cnt].astype(np.float32)).astype(npe4)
        bias = np.full(s_half, MASK_NEG, dtype=np.float32)
        bias[:cnt] = np.where(mask[b][ix] == 0, np.float32(MASK_NEG), 0.0)
        in_maps.append(
            {
                "hs8T": a8,
                "hsb8T": b8,
                "wk8": wk8c,
                "wvc8": wvcc,
                "wvd8": wvdc,
                "qtH": qtc,
                "mbT": np.ascontiguousarray(bias.reshape(s_half // 128, 128).T),
            }
        )
    return in_maps, s_half


def time_device(inputs_np, reps=8):
    """Dev-only helper (not used by grading): time repeated NEFF executions
    with inputs resident on device. Mirrors bass2jax.run_bass_via_pjrt's
    multi-core path; each wall time includes one axon execute round-trip."""
    import time

    import jax
    from jax.experimental.shard_map import shard_map
    from jax.sharding import Mesh, NamedSharding, PartitionSpec

    import concourse.mybir as mybir_
    from concourse import bass2jax

    in_maps, s_half = _prep_in_maps(
        np.asarray(inputs_np["hidden_states"], np.float32),
        np.asarray(inputs_np["attention_mask"]),
        np.asarray(inputs_np["memory_slots"], np.float32),
        np.asarray(inputs_np["Wq"], np.float32),
        np.asarray(inputs_np["Wk"], np.float32),
        np.asarray(inputs_np["Wv"], np.float32),
        np.asarray(inputs_np["Wo"], np.float32),
    )
    nc = _get_module(s_half)
    bass2jax.install_neuronx_cc_hook()

    in_names, out_names, out_avals, zero_outs = [], [], [], []
    has_partition = False
    for alloc in nc.m.functions[0].allocations:
        if not isinstance(alloc, mybir_.MemoryLocationSet):
            continue
        name = alloc.memorylocations[0].name
        if alloc.kind == "ExternalInput":
            if name == "partition_id":
                has_partition = True
                continue
            in_names.append(name)
        elif alloc.kind == "ExternalOutput":
            out_names.append(name)
            shape = tuple(alloc.tensor_shape)
            dtype = mybir_.dt.np(alloc.dtype)
            out_avals.append(jax.core.ShapedArray(shape, dtype))
            zero_outs.append(np.zeros(shape, dtype))
    n_params = len(in_names)
    n_outs = len(out_avals)
    all_names = in_names + (["partition_id"] if has_partition else []) + out_names

    def _body(*args):
        operands = list(args[:n_params])
        if has_partition:
            operands.append(bass2jax.partition_id_tensor())
        operands += list(args[n_params:])
        outs = bass2jax._bass_exec_p.bind(
            *operands,
            out_avals=tuple(out_avals),
            in_names=tuple(all_names),
            out_names=tuple(out_names),
            lowering_input_output_aliases=(),
            sim_require_finite=True,
            sim_require_nnan=True,
            nc=nc,
        )
        return tuple(outs)

    devices = jax.devices()[:N_CORES]
    mesh = Mesh(np.asarray(devices), ("core",))
    spec = PartitionSpec("core")
    sharded = jax.jit(
        shard_map(
            _body,
            mesh=mesh,
            in_specs=(spec,) * (n_params + n_outs),
            out_specs=(spec,) * n_outs,
            check_rep=False,
        ),
        donate_argnums=tuple(range(n_params, n_params + n_outs)),
        keep_unused=True,
    )
    concat_in = [
        np.concatenate([np.asarray(in_maps[c][nm]) for c in range(N_CORES)], axis=0)
        for nm in in_names
    ]
    sh = NamedSharding(mesh, spec)
    dev_in = [jax.device_put(a, sh) for a in concat_in]
    jax.block_until_ready(dev_in)

    times = []
    for _ in range(reps):
        zeros = [np.zeros((N_CORES * z.shape[0], *z.shape[1:]), z.dtype)
                 for z in zero_outs]
        dz = [jax.device_put(z, sh) for z in zeros]
        jax.block_until_ready(dz)
        t0 = time.perf_counter()
        out = sharded(*dev_in, *dz)
        jax.block_until_ready(out)
        times.append(time.perf_counter() - t0)
    return times


def kernel(hidden_states, attention_mask, memory_slots, Wq, Wk, Wv, Wo):
    global LAST_RESULT
    hs = np.asarray(hidden_states, dtype=np.float32)
    mask = np.asarray(attention_mask)
    ms = np.asarray(memory_slots, dtype=np.float32)
    Wq = np.asarray(Wq, dtype=np.float32)
    Wk = np.asarray(Wk, dtype=np.float32)
    Wv = np.asarray(Wv, dtype=np.float32)
    Wo = np.asarray(Wo, dtype=np.float32)

    in_maps, s_half = _prep_in_maps(hs, mask, ms, Wq, Wk, Wv, Wo)
    nc = _get_module(s_half)

    kwargs = {}
    if TRACE:
        kwargs = {"trace": True}
        if TRACE_CORES is not None:
            kwargs["trace_cores"] = TRACE_CORES
    res = run_bass_kernel_spmd(nc, in_maps, core_ids=list(range(N_CORES)), **kwargs)
    LAST_RESULT = res

    # combine the two row-halves per batch: sum unnormalized o and the
    # softmax partials, divide, then o_proj in f32 on the host.
    y = np.empty((B, SLOTS, HID), np.float32)
    for b in range(B):
        t = np.zeros((SLOTS, HEADS, HD + 1), np.float64)
        for r in range(HALVES):
            t += res.results[HALVES * b + r]["oc"].reshape(SLOTS, HEADS, HD + 1)
        o = (t[:, :, :HD] / t[:, :, HD : HD + 1] * KINV).reshape(SLOTS, BD)
        y[b] = (o @ Wo.T.astype(np.float64)).astype(np.float32)
    return np.ascontiguousarray(y)
